# revision 21
# baseline (speedup 1.0000x reference)
"""Trainium2 Bass kernel for quantized 3x3 conv (CWTConv2D).

Reference computation:
    x_q = round(x)                      # [B,512,512] f32, round-half-even
    k_q = clip(round(kernel_w), -1, 1)  # [32,3,3]
    out[b,h,w,f] = relu(sum_{kh,kw} x_q[b,h+kh,w+kw] * k_q[f,2-kh,2-kw]
                        + round(bias[f]))            # [B,510,510,32]

All arithmetic is exact small-integer math (|x_q| <= ~6, |out| <= ~54):
x_q and the weights are exact in fp8_e4m3 (integers to 16), matmul
accumulates in f32 PSUM, and the output is stored as uint8 (exact for
0..255 after relu); the host upcasts to f32. uint8 output cuts HBM
write traffic 4x vs f32; fp8 scratch halves the gather round-trip.

Per-core structure (pure data parallel, 4 images/core), pipelined at
half-image granularity (8 halves of 256 rows = 16 g4 groups each):
  1. stage image to SBUF packed 4 rows/partition (one 8KB-line DMA),
     round to integer fp8 with one DVE tensor_scalar
     ((x + 1.5*2^23) - 1.5*2^23 == rint(x), exact, fp32 internal).
  2. write the rounded image to a DRAM scratch xqd3 THREE times, one
     copy per kw shift (3 DMAs): flat elem (3r+kw)*RP + w holds
     x_q[r, w+kw]. This makes the (i,kw) pair a single affine
     dimension u=3i+kw of stride RP in DRAM, so each half's Toeplitz
     gather is 4 DMAs (one per strip s), 3-dim APs on both sides:
     rh[32s+u, g4l*512+w] <- xqd3[(3*(256h+16g4l+4s)+u)*RP + w].
  3. per g4: four K=18 fp8 matmuls (s=0..3) at tile_position (32s, 0)
     (concurrent PE row-strips, block-Toeplitz lhsT with kw baked in)
     into a 4-bank PSUM quad [128, 2048] f32.
  4. fused bias+relu+uint8 evict per quad, strictly alternating
     ScalarE/VectorE (Bresenham 69:59) so both PSUM readers run in
     parallel continuously — this is the throughput bound (~1 elem/
     lane/cycle each, ~140us/core).
  5. per half raw dump [128, 32KB] uint8, alternating the two HWDGE
     rings (ACT/SP), issued 2 quads into the next half so the issuing
     engine never stalls waiting for the other evictor.
"""

import numpy as np
import ml_dtypes

import bass_rust
from bass_rust import add_dep_helper
from concourse import bass, mybir
from concourse.tile import TileContext
from concourse.vector_clock import ScopedClock
from concourse.bass_utils import run_bass_kernel_spmd

N_CORES = 8
B, H, W = 32, 512, 512
F = 32
B_SHARD = B // N_CORES          # 4 images per core
H_OUT, W_OUT = H - 2, W - 2     # 510, 510
G4H = 16                        # g4 groups per half-image
RP = 520                        # xq SBUF row-slot pitch (fp8 elements)
RPD = 512                       # xqd3 DRAM kw-copy pitch: full rows contiguous
MAGIC = 12582912.0              # 1.5 * 2**23: (x + MAGIC) - MAGIC == rint(x)
ACT_SHARE = 137                 # of 256 half-quads evicted on ScalarE (rest DVE)
N_EVICT = 256

_F8 = mybir.dt.float8e4
_F32 = mybir.dt.float32
_U8 = mybir.dt.uint8


def _patch_drain_waits():
    """walrus in this container only accepts ONE sem-wait per SP CTRL
    instruction; Tile's kernel-tail drain carries several. Split the
    extras onto dedicated single-wait nops."""
    if getattr(TileContext, "_drain_waits_patched", False):
        return

    def _drain_and_barrier(self, tick_clock, wait_clock):
        nc = self.nc
        drain_inst = nc.sync.drain()
        wait_clock.add_sem_waits(
            drain_inst.ins, ScopedClock({None: tick_clock.global_clock})
        )
        si = drain_inst.ins.sync_info
        waits = list(si.on_wait)
        if len(waits) > 1:
            si.on_wait = waits[:1]
            for w in waits[1:]:
                nop = nc.sync.nop(nofuse=True, hint="drain_wait_spill")
                nop.ins.sync_info = bass_rust.SyncInfo(on_wait=[w], on_update=[])
        nc.all_engine_barrier()
        popped = nc._tile_sem_poison_stack.pop()
        assert popped is self._sem_poison
        nc.clear_and_free_semaphores(list(self.sems.allocated().values()))
        nc.all_engine_barrier()

    TileContext._drain_and_barrier = _drain_and_barrier
    TileContext._drain_waits_patched = True


def _split_multi_waits(nc, max_waits=1):
    """walrus here rejects instructions carrying more than one sem-wait
    (any engine, incl. DMA). Hoist extras onto single-wait nops placed
    immediately before, on the same engine (per-engine order preserved)."""
    counter = [0]
    for fn in nc.m.functions:
        for block in fn.blocks:
            new_insts = []
            for inst in block.instructions:
                si = inst.sync_info
                if si is not None and len(si.on_wait) > max_waits:
                    waits = list(si.on_wait)
                    for w in waits[:-max_waits]:
                        counter[0] += 1
                        nop = mybir.InstNoOp(
                            name=f"waitspill-{counter[0]}",
                            engine=inst.engine,
                            sync_info=mybir.SyncInfo(on_wait=[w], on_update=[]),
                            bass_nofuse=True,
                        )
                        new_insts.append(nop)
                    si.on_wait = waits[-max_waits:]
                new_insts.append(inst)
            block.instructions = new_insts


def _is_act_quad(qi):
    """Bresenham split of N_EVICT half-quads into ACT_SHARE ScalarE / DVE."""
    return (qi + 1) * ACT_SHARE // N_EVICT > qi * ACT_SHARE // N_EVICT


def _build_program():
    _patch_drain_waits()
    nc = bass.Bass()

    x_in = nc.declare_dram_parameter("x", [B_SHARD, H, W], _F32, isOutput=False)
    w_in = nc.declare_dram_parameter("w", [18, 128], _F8, isOutput=False)
    b_in = nc.declare_dram_parameter("bias", [128, 1], _F32, isOutput=False)
    y_out = nc.declare_dram_parameter(
        "y", [B_SHARD, 2, 128, 16 * 2048], _U8, isOutput=True
    )

    relu = mybir.ActivationFunctionType.Relu
    add_op = mybir.AluOpType.add
    sub_op = mybir.AluOpType.subtract
    max_op = mybir.AluOpType.max

    with TileContext(nc) as tc:
        with (
            tc.tile_pool(name="const", bufs=1) as cpool,
            tc.tile_pool(name="stage", bufs=2) as stage_pool,
            tc.tile_pool(name="xq", bufs=2) as xq_pool,
            tc.tile_pool(name="xqd", bufs=2, space="DRAM") as xqd_pool,
            tc.tile_pool(name="rh", bufs=2) as rh_pool,
            tc.tile_pool(name="outb", bufs=2) as outb_pool,
            tc.tile_pool(name="psum", bufs=4, space="PSUM") as psum_pool,
        ):
            # consts on the ACT ring so the SP ring starts with stage(0)
            w_tile = cpool.tile([128, 128], _F8)
            for s in range(4):
                nc.scalar.dma_start(out=w_tile[32 * s : 32 * s + 18, :], in_=w_in[:])
            bias_tile = cpool.tile([128, 1], _F32)
            nc.scalar.dma_start(out=bias_tile[:], in_=b_in[:])
            zrow = cpool.tile([1, 6 * RPD], _F8)
            nc.gpsimd.memset(zrow[:], 0.0)

            state = {"qi": 0, "pending_dump": None}
            gathers_by_img = []     # for xqd WAR (slot reused by image b+2)
            wrs_by_img = []         # for xq WAR (round b reuses slot of b-2)
            last_mm_by_half = []    # for rh WAR (slot reused by half hh+2)
            xqd_by_img = []

            def prep(b):
                """stage + round + fp8 triple write-back for image b.

                Images 0/1 round on DVE (fast, nothing queued yet); images
                2/3 round on the otherwise-idle GPSIMD (slow but fully
                overlapped), keeping the round out of DVE's evict stream."""
                stage = stage_pool.tile([128, 2048], _F32)
                nc.sync.dma_start(
                    out=stage[:],
                    in_=x_in[b].rearrange("(p j) w -> p (j w)", p=128),
                )
                xq = xq_pool.tile([128, 4 * RP], _F8)
                reng = nc.vector if b < 2 else nc.gpsimd
                rnd = reng.tensor_scalar(
                    out=xq.rearrange("p (j w) -> p j w", w=RP)[:, :, 0:W],
                    in0=stage.rearrange("p (j w) -> p j w", w=W),
                    scalar1=MAGIC,
                    scalar2=MAGIC,
                    op0=add_op,
                    op1=sub_op,
                )
                # WAR: xq slot (bufs=2) was read by image b-2's xqd writes
                if b >= 2:
                    for wr in wrs_by_img[b - 2]:
                        add_dep_helper(rnd.ins, wr, sync=True, reason="xq WAR")
                # xqd3: flat elem (3r+kw)*RPD + w == x_q[r, w+kw]
                xqd = xqd_pool.tile([1542, RPD], _F8)
                xqd_by_img.append(xqd)
                fxq = xq[0:1, :].ap[0][0]
                wrs = []
                for kw in range(3):
                    # row 4p+j lives at xq[p, j*RP:...]; shift kw via src offset
                    src = bass.AP(xq.tensor, kw, [[fxq, 128], [RP, 4], [1, W]])
                    dst = bass.AP(
                        xqd.tensor,
                        kw * RPD,
                        [[12 * RPD, 128], [3 * RPD, 4], [1, W]],
                    )
                    wr = nc.sync.dma_start(out=dst, in_=src)
                    # RAW: reads xq written by the round
                    add_dep_helper(wr.ins, rnd.ins, sync=True, reason="RAW xq")
                    wrs.append(wr.ins)
                # zero rows 512-513 (all kw copies): the last groups' matmuls
                # read them with zero weights; 0 * NaN-junk would poison
                # valid outputs
                wz = nc.sync.dma_start(
                    out=bass.AP(
                        xqd.tensor, 1536 * RPD, [[6 * RPD, 1], [1, 6 * RPD]]
                    ),
                    in_=zrow[:],
                )
                wrs.append(wz.ins)
                # WAR: this xqd slot (bufs=2) was read by image b-2's gathers
                if b >= 2:
                    for g in gathers_by_img[b - 2]:
                        for wr in wrs:
                            add_dep_helper(wr, g, sync=True, reason="xqd WAR")
                gathers_by_img.append([])
                wrs_by_img.append(wrs)

            rh_by_half = {}

            def emit_gathers(hh):
                """4 gather DMAs (one per strip) for half hh; 1-half lookahead
                keeps them ahead of dump transfers in SP-ring FIFO order."""
                b, h = hh // 2, hh % 2
                xqd = xqd_by_img[b]
                wrs = wrs_by_img[b]
                rh = rh_pool.tile([128, G4H * W], _F8)
                fwd = rh[0:1, :].ap[0][0]
                gathers = []
                for s in range(4):
                    src = bass.AP(
                        xqd.tensor,
                        3 * (256 * h + 4 * s) * RPD,
                        [[RPD, 18], [48 * RPD, G4H], [1, W]],
                    )
                    dst = bass.AP(
                        rh.tensor, 32 * s * fwd, [[fwd, 18], [W, G4H], [1, W]]
                    )
                    gathers.append(nc.sync.dma_start(out=dst, in_=src).ins)
                for g in gathers:
                    # RAW: gather reads xqd written by prep(b)
                    for wr in wrs:
                        add_dep_helper(g, wr, sync=True, reason="RAW xqd")
                    # WAR: rh slot was read by half hh-2's matmuls
                    if hh >= 2:
                        add_dep_helper(
                            g, last_mm_by_half[hh - 2], sync=True, reason="rh WAR"
                        )
                    gathers_by_img[b].append(g)
                rh_by_half[hh] = (rh, gathers)

            def emit_half(b, h):
                hh = 2 * b + h
                if hh + 1 < 2 * B_SHARD:
                    emit_gathers(hh + 1)
                # prep images 2/3 early (at hh 0/2) so their gpsimd rounds
                # fully overlap compute; placed after emit_gathers so the
                # xqd-WAR gather list for image b is complete and the SP
                # program order stays deadlock-free
                if hh in (0, 2) and hh // 2 + 2 < B_SHARD:
                    prep(hh // 2 + 2)
                rh, gathers = rh_by_half.pop(hh)
                # PE-order join: matmuls of this half follow this nop
                pe_join = nc.tensor.nop(nofuse=True, hint="rh_ready")
                for g in gathers:
                    add_dep_helper(pe_join.ins, g, sync=True, reason="rh RAW")

                # --- matmuls + evict in 2-bank half-quads (4 PSUM slots) ---
                outb = outb_pool.tile([128, 16 * 2048], _U8)
                last_mm = None
                last_half = hh == 2 * B_SHARD - 1
                for g4l in range(G4H):
                    for t in range(2):          # strip pairs (0,1) and (2,3)
                        ps = psum_pool.tile([128, 1024], _F32)
                        for si in range(2):
                            s = 2 * t + si
                            mm = nc.tensor.matmul(
                                ps[:, si * W : (si + 1) * W],
                                w_tile[32 * s : 32 * s + 18, :],
                                rh[32 * s : 32 * s + 18, g4l * W : (g4l + 1) * W],
                                start=True,
                                stop=True,
                                tile_position=(32 * s, 0),
                            )
                            last_mm = mm.ins
                        dst = outb[
                            :, g4l * 2048 + t * 1024 : g4l * 2048 + (t + 1) * 1024
                        ]
                        if _is_act_quad(state["qi"]):
                            nc.scalar.activation(
                                dst, ps[:], relu, bias=bias_tile[:], scale=1.0
                            )
                        else:
                            nc.vector.tensor_scalar(
                                out=dst,
                                in0=ps[:],
                                scalar1=bias_tile[:],
                                scalar2=0.0,
                                op0=add_op,
                                op1=max_op,
                            )
                        state["qi"] += 1
                    if g4l == 1 and state["pending_dump"] is not None:
                        eng, dump_dst, dump_src = state["pending_dump"]
                        eng.dma_start(out=dump_dst, in_=dump_src)
                        state["pending_dump"] = None
                    if last_half and g4l in (3, 7, 11, 13, 15):
                        # final half: dump in shrinking chunks to cut the tail
                        bounds = {3: (0, 8192), 7: (8192, 16384),
                                  11: (16384, 24576), 13: (24576, 28672),
                                  15: (28672, 32768)}
                        lo, hi = bounds[g4l]
                        nc.sync.dma_start(
                            out=y_out[b, h][:, lo:hi],
                            in_=outb[:, lo:hi],
                        )
                last_mm_by_half.append(last_mm)
                if not last_half:
                    # alternate dump ring: even halves ACT, odd halves SP
                    eng = nc.scalar if hh % 2 == 0 else nc.sync
                    state["pending_dump"] = (eng, y_out[b, h], outb[:])

            prep(0)
            prep(1)
            emit_gathers(0)
            for b in range(B_SHARD):
                for h in range(2):
                    emit_half(b, h)
    _split_multi_waits(nc)
    return nc


_PROGRAM = None


def _get_program():
    global _PROGRAM
    if _PROGRAM is None:
        _PROGRAM = _build_program()
    return _PROGRAM


def _host_weights(kernel_w):
    """Block-Toeplitz lhsT [18, 128]: lhsT[3i+kw, 32r+f] = W_eff[i-r, kw, f]
    where W_eff[kh,kw,f] = k_q[f, 2-kh, 2-kw] (true-conv spatial flip)."""
    k_q = np.clip(np.round(kernel_w.astype(np.float64)), -1.0, 1.0)
    w_eff = k_q[:, ::-1, ::-1].transpose(1, 2, 0)  # [kh, kw, f]
    lhsT = np.zeros((6, 3, 128), np.float64)
    for i in range(6):
        for kw in range(3):
            for r in range(4):
                kh = i - r
                if 0 <= kh <= 2:
                    lhsT[i, kw, 32 * r : 32 * r + 32] = w_eff[kh, kw, :]
    return lhsT.reshape(18, 128).astype(ml_dtypes.float8_e4m3)


def kernel(x, kernel_w, biases):
    x = np.asarray(x, np.float32)
    lhsT = _host_weights(np.asarray(kernel_w))
    bias_r = np.round(np.asarray(biases, np.float64)).astype(np.float32)
    bias_col = np.tile(bias_r, 4).reshape(128, 1)

    nc = _get_program()
    in_maps = [
        {
            "x": np.ascontiguousarray(x[c * B_SHARD : (c + 1) * B_SHARD]),
            "w": lhsT,
            "bias": bias_col,
        }
        for c in range(N_CORES)
    ]
    res = run_bass_kernel_spmd(nc, in_maps, list(range(N_CORES)))

    out = np.empty((B, H_OUT, W_OUT, F), np.float32)
    for c in range(N_CORES):
        y = res.results[c]["y"]  # [B_SHARD, 2, 128, 32768] uint8
        # [b, half, (r f), (g4l s w)] -> h = 256*half + 16*g4l + 4*s + r
        y7 = y.reshape(B_SHARD, 2, 4, F, 16, 4, W)
        nhwc = y7.transpose(0, 1, 4, 5, 2, 6, 3).reshape(B_SHARD, 512, W, F)
        out[c * B_SHARD : (c + 1) * B_SHARD] = nhwc[:, :H_OUT, :W_OUT, :]
    return out


# revision 25
# speedup vs baseline: 1.0924x; 1.0924x over previous
"""Trainium2 Bass kernel for quantized 3x3 conv (CWTConv2D).

Reference computation:
    x_q = round(x)                      # [B,512,512] f32, round-half-even
    k_q = clip(round(kernel_w), -1, 1)  # [32,3,3]
    out[b,h,w,f] = relu(sum_{kh,kw} x_q[b,h+kh,w+kw] * k_q[f,2-kh,2-kw]
                        + round(bias[f]))            # [B,510,510,32]

All arithmetic is exact small-integer math (|x_q| <= ~6, |out| <= ~54):
x_q and the weights are exact in fp8_e4m3 (integers to 16), matmul
accumulates in f32 PSUM, and the output is stored as uint8 (exact for
0..255 after relu); the host upcasts to f32. uint8 output cuts HBM
write traffic 4x vs f32; fp8 scratch halves the gather round-trip.

Per-core structure (pure data parallel, 4 images/core), pipelined at
half-image granularity (8 halves of 256 rows = 16 g4 groups each):
  1. stage image to SBUF packed 4 rows/partition (one 8KB-line DMA),
     round to integer fp8 with one DVE tensor_scalar
     ((x + 1.5*2^23) - 1.5*2^23 == rint(x), exact, fp32 internal).
  2. write the rounded image to a DRAM scratch xqd3 THREE times, one
     copy per kw shift (3 DMAs): flat elem (3r+kw)*RP + w holds
     x_q[r, w+kw]. This makes the (i,kw) pair a single affine
     dimension u=3i+kw of stride RP in DRAM, so each half's Toeplitz
     gather is 4 DMAs (one per strip s), 3-dim APs on both sides:
     rh[32s+u, g4l*512+w] <- xqd3[(3*(256h+16g4l+4s)+u)*RP + w].
  3. per g4: four K=18 fp8 matmuls (s=0..3) at tile_position (32s, 0)
     (concurrent PE row-strips, block-Toeplitz lhsT with kw baked in)
     into a 4-bank PSUM quad [128, 2048] f32.
  4. fused bias+relu+uint8 evict per quad, strictly alternating
     ScalarE/VectorE (Bresenham 69:59) so both PSUM readers run in
     parallel continuously — this is the throughput bound (~1 elem/
     lane/cycle each, ~140us/core).
  5. per half raw dump [128, 32KB] uint8, alternating the two HWDGE
     rings (ACT/SP), issued 2 quads into the next half so the issuing
     engine never stalls waiting for the other evictor.
"""

import numpy as np
import ml_dtypes

import bass_rust
from bass_rust import add_dep_helper
from concourse import bass, mybir
from concourse.tile import TileContext
from concourse.vector_clock import ScopedClock
from concourse.bass_utils import run_bass_kernel_spmd

N_CORES = 8
B, H, W = 32, 512, 512
F = 32
B_SHARD = B // N_CORES          # 4 images per core
H_OUT, W_OUT = H - 2, W - 2     # 510, 510
G4H = 16                        # g4 groups per half-image
RP = 520                        # xq SBUF row-slot pitch (fp8 elements)
RPD = 512                       # xqd3 DRAM kw-copy pitch: full rows contiguous
MAGIC = 12582912.0              # 1.5 * 2**23: (x + MAGIC) - MAGIC == rint(x)
ACT_SHARE = 137                 # of 256 half-quads evicted on ScalarE (rest DVE)
N_EVICT = 256

_F8 = mybir.dt.float8e4
_F32 = mybir.dt.float32
_U8 = mybir.dt.uint8


def _patch_drain_waits():
    """walrus in this container only accepts ONE sem-wait per SP CTRL
    instruction; Tile's kernel-tail drain carries several. Split the
    extras onto dedicated single-wait nops."""
    if getattr(TileContext, "_drain_waits_patched", False):
        return

    def _drain_and_barrier(self, tick_clock, wait_clock):
        nc = self.nc
        drain_inst = nc.sync.drain()
        wait_clock.add_sem_waits(
            drain_inst.ins, ScopedClock({None: tick_clock.global_clock})
        )
        si = drain_inst.ins.sync_info
        waits = list(si.on_wait)
        if len(waits) > 1:
            si.on_wait = waits[:1]
            for w in waits[1:]:
                nop = nc.sync.nop(nofuse=True, hint="drain_wait_spill")
                nop.ins.sync_info = bass_rust.SyncInfo(on_wait=[w], on_update=[])
        nc.all_engine_barrier()
        popped = nc._tile_sem_poison_stack.pop()
        assert popped is self._sem_poison
        nc.clear_and_free_semaphores(list(self.sems.allocated().values()))
        nc.all_engine_barrier()

    TileContext._drain_and_barrier = _drain_and_barrier
    TileContext._drain_waits_patched = True


def _split_multi_waits(nc, max_waits=1):
    """walrus here rejects instructions carrying more than one sem-wait
    (any engine, incl. DMA). Hoist extras onto single-wait nops placed
    immediately before, on the same engine (per-engine order preserved)."""
    counter = [0]
    for fn in nc.m.functions:
        for block in fn.blocks:
            new_insts = []
            for inst in block.instructions:
                si = inst.sync_info
                if si is not None and len(si.on_wait) > max_waits:
                    waits = list(si.on_wait)
                    for w in waits[:-max_waits]:
                        counter[0] += 1
                        nop = mybir.InstNoOp(
                            name=f"waitspill-{counter[0]}",
                            engine=inst.engine,
                            sync_info=mybir.SyncInfo(on_wait=[w], on_update=[]),
                            bass_nofuse=True,
                        )
                        new_insts.append(nop)
                    si.on_wait = waits[-max_waits:]
                new_insts.append(inst)
            block.instructions = new_insts


def _is_act_quad(qi):
    """Bresenham split of N_EVICT half-quads into ACT_SHARE ScalarE / DVE."""
    return (qi + 1) * ACT_SHARE // N_EVICT > qi * ACT_SHARE // N_EVICT


def _build_program():
    _patch_drain_waits()
    nc = bass.Bass()

    x_in = nc.declare_dram_parameter("x", [B_SHARD, H, W], _F32, isOutput=False)
    w_in = nc.declare_dram_parameter("w", [18, 128], _F8, isOutput=False)
    b_in = nc.declare_dram_parameter("bias", [128, 1], _F32, isOutput=False)
    y_out = nc.declare_dram_parameter(
        "y", [B_SHARD, 2, 128, 16 * 2048], _U8, isOutput=True
    )

    relu = mybir.ActivationFunctionType.Relu
    add_op = mybir.AluOpType.add
    sub_op = mybir.AluOpType.subtract
    max_op = mybir.AluOpType.max

    with TileContext(nc) as tc:
        with (
            tc.tile_pool(name="const", bufs=1) as cpool,
            tc.tile_pool(name="stage", bufs=4) as stage_pool,
            tc.tile_pool(name="xq", bufs=2) as xq_pool,
            tc.tile_pool(name="xqd", bufs=2, space="DRAM") as xqd_pool,
            tc.tile_pool(name="rh", bufs=2) as rh_pool,
            tc.tile_pool(name="outb", bufs=2) as outb_pool,
            tc.tile_pool(name="psum", bufs=4, space="PSUM") as psum_pool,
        ):
            # consts on the ACT ring so the SP ring starts with stage(0)
            w_tile = cpool.tile([128, 128], _F8)
            for s in range(4):
                nc.scalar.dma_start(out=w_tile[32 * s : 32 * s + 18, :], in_=w_in[:])
            bias_tile = cpool.tile([128, 1], _F32)
            nc.scalar.dma_start(out=bias_tile[:], in_=b_in[:])
            zrow = cpool.tile([1, 6 * RPD], _F8)
            nc.gpsimd.memset(zrow[:], 0.0)

            state = {"qi": 0, "pending_dump": None}
            gathers_by_img = []     # for xqd WAR (slot reused by image b+2)
            wrs_by_img = []         # for xq WAR (round b reuses slot of b-2)
            last_mm_by_half = []    # for rh WAR (slot reused by half hh+2)
            xqd_by_img = []
            stage_by_img = []

            def stage_load(b):
                """wait-free 1MB stage DMA; all four issued up front so the
                SP ring never stalls on them and rounds are never input-bound."""
                stage = stage_pool.tile([128, 2048], _F32)
                nc.sync.dma_start(
                    out=stage[:],
                    in_=x_in[b].rearrange("(p j) w -> p (j w)", p=128),
                )
                stage_by_img.append(stage)

            def round_write(b):
                """DVE round (1.2us, fp8 out) + fp8 triple write-back."""
                stage = stage_by_img[b]
                xq = xq_pool.tile([128, 4 * RP], _F8)
                rnd = nc.vector.tensor_scalar(
                    out=xq.rearrange("p (j w) -> p j w", w=RP)[:, :, 0:W],
                    in0=stage.rearrange("p (j w) -> p j w", w=W),
                    scalar1=MAGIC,
                    scalar2=MAGIC,
                    op0=add_op,
                    op1=sub_op,
                )
                # WAR: xq slot (bufs=2) was read by image b-2's xqd writes
                if b >= 2:
                    for wr in wrs_by_img[b - 2]:
                        add_dep_helper(rnd.ins, wr, sync=True, reason="xq WAR")
                # xqd3: flat elem (3r+kw)*RPD + w == x_q[r, w+kw]
                xqd = xqd_pool.tile([1542, RPD], _F8)
                xqd_by_img.append(xqd)
                fxq = xq[0:1, :].ap[0][0]
                wrs = []
                for kw in range(3):
                    # row 4p+j lives at xq[p, j*RP:...]; shift kw via src offset
                    src = bass.AP(xq.tensor, kw, [[fxq, 128], [RP, 4], [1, W]])
                    dst = bass.AP(
                        xqd.tensor,
                        kw * RPD,
                        [[12 * RPD, 128], [3 * RPD, 4], [1, W]],
                    )
                    wr = nc.sync.dma_start(out=dst, in_=src)
                    # RAW: reads xq written by the round
                    add_dep_helper(wr.ins, rnd.ins, sync=True, reason="RAW xq")
                    wrs.append(wr.ins)
                # zero rows 512-513 (all kw copies): the last groups' matmuls
                # read them with zero weights; 0 * NaN-junk would poison
                # valid outputs
                wz = nc.sync.dma_start(
                    out=bass.AP(
                        xqd.tensor, 1536 * RPD, [[6 * RPD, 1], [1, 6 * RPD]]
                    ),
                    in_=zrow[:],
                )
                wrs.append(wz.ins)
                # WAR: this xqd slot (bufs=2) was read by image b-2's gathers
                if b >= 2:
                    for g in gathers_by_img[b - 2]:
                        for wr in wrs:
                            add_dep_helper(wr, g, sync=True, reason="xqd WAR")
                gathers_by_img.append([])
                wrs_by_img.append(wrs)

            rh_by_half = {}

            def emit_gathers(hh):
                """4 gather DMAs (one per strip) for half hh; 1-half lookahead
                keeps them ahead of dump transfers in SP-ring FIFO order."""
                b, h = hh // 2, hh % 2
                xqd = xqd_by_img[b]
                wrs = wrs_by_img[b]
                rh = rh_pool.tile([128, G4H * W], _F8)
                fwd = rh[0:1, :].ap[0][0]
                gathers = []
                for s in range(4):
                    src = bass.AP(
                        xqd.tensor,
                        3 * (256 * h + 4 * s) * RPD,
                        [[RPD, 18], [48 * RPD, G4H], [1, W]],
                    )
                    dst = bass.AP(
                        rh.tensor, 32 * s * fwd, [[fwd, 18], [W, G4H], [1, W]]
                    )
                    gathers.append(nc.sync.dma_start(out=dst, in_=src).ins)
                for g in gathers:
                    # RAW: gather reads xqd written by prep(b)
                    for wr in wrs:
                        add_dep_helper(g, wr, sync=True, reason="RAW xqd")
                    # WAR: rh slot was read by half hh-2's matmuls
                    if hh >= 2:
                        add_dep_helper(
                            g, last_mm_by_half[hh - 2], sync=True, reason="rh WAR"
                        )
                    gathers_by_img[b].append(g)
                rh_by_half[hh] = (rh, gathers)

            def emit_half(b, h):
                hh = 2 * b + h
                if hh + 1 < 2 * B_SHARD:
                    emit_gathers(hh + 1)
                # round+write images 2/3 early (at hh 0/2); placed after
                # emit_gathers so the xqd-WAR gather list for the reused
                # slot is complete and the SP program order stays
                # deadlock-free
                if hh in (0, 2) and hh // 2 + 2 < B_SHARD:
                    round_write(hh // 2 + 2)
                rh, gathers = rh_by_half.pop(hh)
                # PE-order join: matmuls of this half follow this nop
                pe_join = nc.tensor.nop(nofuse=True, hint="rh_ready")
                for g in gathers:
                    add_dep_helper(pe_join.ins, g, sync=True, reason="rh RAW")

                # --- matmuls + evict in 2-bank half-quads (4 PSUM slots) ---
                outb = outb_pool.tile([128, 16 * 2048], _U8)
                last_mm = None
                last_half = hh == 2 * B_SHARD - 1
                for g4l in range(G4H):
                    for t in range(2):          # strip pairs (0,1) and (2,3)
                        ps = psum_pool.tile([128, 1024], _F32)
                        for si in range(2):
                            s = 2 * t + si
                            mm = nc.tensor.matmul(
                                ps[:, si * W : (si + 1) * W],
                                w_tile[32 * s : 32 * s + 18, :],
                                rh[32 * s : 32 * s + 18, g4l * W : (g4l + 1) * W],
                                start=True,
                                stop=True,
                                tile_position=(32 * s, 0),
                            )
                            last_mm = mm.ins
                        dst = outb[
                            :, g4l * 2048 + t * 1024 : g4l * 2048 + (t + 1) * 1024
                        ]
                        if _is_act_quad(state["qi"]):
                            nc.scalar.activation(
                                dst, ps[:], relu, bias=bias_tile[:], scale=1.0
                            )
                        else:
                            nc.vector.tensor_scalar(
                                out=dst,
                                in0=ps[:],
                                scalar1=bias_tile[:],
                                scalar2=0.0,
                                op0=add_op,
                                op1=max_op,
                            )
                        state["qi"] += 1
                    if g4l == 1 and state["pending_dump"] is not None:
                        eng, dump_dst, dump_src = state["pending_dump"]
                        eng.dma_start(out=dump_dst, in_=dump_src)
                        state["pending_dump"] = None
                    if last_half and g4l in (3, 7, 11, 13, 15):
                        # final half: dump in shrinking chunks to cut the tail
                        bounds = {3: (0, 8192), 7: (8192, 16384),
                                  11: (16384, 24576), 13: (24576, 28672),
                                  15: (28672, 32768)}
                        lo, hi = bounds[g4l]
                        nc.sync.dma_start(
                            out=y_out[b, h][:, lo:hi],
                            in_=outb[:, lo:hi],
                        )
                last_mm_by_half.append(last_mm)
                if not last_half:
                    # alternate dump ring: even halves ACT, odd halves SP
                    eng = nc.scalar if hh % 2 == 0 else nc.sync
                    state["pending_dump"] = (eng, y_out[b, h], outb[:])

            for b in range(B_SHARD):
                stage_load(b)
            round_write(0)
            emit_gathers(0)
            round_write(1)
            for b in range(B_SHARD):
                for h in range(2):
                    emit_half(b, h)
    _split_multi_waits(nc)
    return nc


_PROGRAM = None


def _get_program():
    global _PROGRAM
    if _PROGRAM is None:
        _PROGRAM = _build_program()
    return _PROGRAM


def _host_weights(kernel_w):
    """Block-Toeplitz lhsT [18, 128]: lhsT[3i+kw, 32r+f] = W_eff[i-r, kw, f]
    where W_eff[kh,kw,f] = k_q[f, 2-kh, 2-kw] (true-conv spatial flip)."""
    k_q = np.clip(np.round(kernel_w.astype(np.float64)), -1.0, 1.0)
    w_eff = k_q[:, ::-1, ::-1].transpose(1, 2, 0)  # [kh, kw, f]
    lhsT = np.zeros((6, 3, 128), np.float64)
    for i in range(6):
        for kw in range(3):
            for r in range(4):
                kh = i - r
                if 0 <= kh <= 2:
                    lhsT[i, kw, 32 * r : 32 * r + 32] = w_eff[kh, kw, :]
    return lhsT.reshape(18, 128).astype(ml_dtypes.float8_e4m3)


def kernel(x, kernel_w, biases):
    x = np.asarray(x, np.float32)
    lhsT = _host_weights(np.asarray(kernel_w))
    bias_r = np.round(np.asarray(biases, np.float64)).astype(np.float32)
    bias_col = np.tile(bias_r, 4).reshape(128, 1)

    nc = _get_program()
    in_maps = [
        {
            "x": np.ascontiguousarray(x[c * B_SHARD : (c + 1) * B_SHARD]),
            "w": lhsT,
            "bias": bias_col,
        }
        for c in range(N_CORES)
    ]
    res = run_bass_kernel_spmd(nc, in_maps, list(range(N_CORES)))

    out = np.empty((B, H_OUT, W_OUT, F), np.float32)
    for c in range(N_CORES):
        y = res.results[c]["y"]  # [B_SHARD, 2, 128, 32768] uint8
        # [b, half, (r f), (g4l s w)] -> h = 256*half + 16*g4l + 4*s + r
        y7 = y.reshape(B_SHARD, 2, 4, F, 16, 4, W)
        nhwc = y7.transpose(0, 1, 4, 5, 2, 6, 3).reshape(B_SHARD, 512, W, F)
        out[c * B_SHARD : (c + 1) * B_SHARD] = nhwc[:, :H_OUT, :W_OUT, :]
    return out
